# revision 1
# baseline (speedup 1.0000x reference)
"""LightGCN-style 3-layer message passing on 8 trn2 NeuronCores.

Math: with deg over dst, dis = deg^-1/2 (0 for isolated), one LGConv layer is
    emb' = dis * segsum_dst(dis[src] * emb[src])
Working variable z_l = dis * emb_l lets each layer be:
    s = segsum_dst(z[src]);  emb' = dis * s;  z' = dis^2 * s
Output = (emb0 + emb1 + emb2 + emb3) / 16 (host adds emb0 and divides).

Distribution: nodes padded to 200704 = 8 * 25088 rows; core m owns dst rows
[m*25088, (m+1)*25088). Edges are partitioned by dst core. Per layer each core:
  - gathers z[src] for its edges from a full replica of z (bf16) in DRAM via
    dma_gather: edges grouped into superchunks of 12 dst-windows (window = 128
    dst rows), bucketed by src range (8 ranges of 25088 so int16 local indices
    fit), one gather call per (superchunk, range) on 4 SWDGE queues
  - segment-sums each 128-edge chunk into its dst window PSUM tile via a
    one-hot matmul (M[e, j] = dst_local[e] == j + 128*wi, built on DVE)
  - scales PSUM by dis (ACT) -> per-layer emb output slice, and by dis^2 (DVE)
    -> z' slice (bf16) which is AllGathered to every core for the next layer
"""

import math
import numpy as np
import ml_dtypes

N_USER = 100000
N_ITEM = 100000
N = N_USER + N_ITEM        # 200000
NCORES = 8
RS = 25088                 # rows per core / per src range
NPAD = RS * NCORES         # 200704
D = 128                    # 64 int + 64 geo features
P = 128
WPC = RS // P              # 196 windows per core
SW = 12                    # windows per superchunk
NSC = math.ceil(WPC / SW)  # 17 (last superchunk has 4 windows)
WGRP = 4                   # windows per PSUM group
L_CAP = 3968               # max idxs per dma_gather call (ring-safe)
BF16 = ml_dtypes.bfloat16

_cache = {}


def _round128(x):
    return ((int(x) + 127) // 128) * 128


def _build_schedule(src, dst):
    """Static per-core edge schedule, identical loop structure for all cores.

    Returns dict with:
      L[s][r]        slots per (superchunk, range) run (same all cores)
      sc_windows[s]  number of dst windows in superchunk s
      idxw[c]        wrapped int16 gather indices per core [128, TOT_SLOTS//16]
      dstw[c]        f32 dst-local-in-superchunk per chunk col [128, TOT_CHUNKS]
      spans[(s,cc)]  list of window indices (sc-local) chunk cc may touch
      chunk_of[s]    list of (global chunk col, range, block-in-run) per sc
      sc_slot_off[s] slot offset of superchunk s
    """
    core = dst // RS
    w_local = (dst - core * RS) // P
    sc = np.minimum(w_local // SW, NSC - 1)
    rng = src // RS

    order = np.lexsort((dst, rng, sc, core))
    core_s, sc_s, rng_s, src_s, dst_s = (
        core[order], sc[order], rng[order], src[order], dst[order])

    key = (core_s * NSC + sc_s) * NCORES + rng_s
    cnt = np.bincount(key, minlength=NCORES * NSC * NCORES).reshape(
        NCORES, NSC, NCORES)
    L = np.maximum(cnt.max(axis=0), 1)
    L = ((L + 127) // 128) * 128  # [NSC, 8]
    assert L.max() <= L_CAP, f"run length {L.max()} exceeds cap"

    run_off = np.zeros((NSC, NCORES), np.int64)   # slot offset of run in sc
    sc_slots = L.sum(axis=1)                      # slots per sc
    sc_slot_off = np.concatenate([[0], np.cumsum(sc_slots)])[:-1]
    for s in range(NSC):
        run_off[s] = np.concatenate([[0], np.cumsum(L[s])])[:-1]
    tot_slots = int(sc_slots.sum())
    tot_chunks = tot_slots // P

    # per-edge slot position (core-local)
    # group start index within the sorted arrays for each (core, sc, r)
    grp_start = np.zeros(NCORES * NSC * NCORES, np.int64)
    k_sorted = key  # already sorted ascending because of lexsort key order
    # compute start of each group
    starts = np.searchsorted(k_sorted, np.arange(NCORES * NSC * NCORES))
    cumcount = np.arange(len(k_sorted)) - starts[k_sorted]
    slot = (sc_slot_off[sc_s] + run_off[sc_s, rng_s] + cumcount)

    idxw = []
    dstw = []
    # wrap: sc-local logical slot i -> (i % 16, sc_off16 + i // 16), tiled x8
    for c in range(NCORES):
        m = core_s == c
        sl = slot[m]
        iv = (src_s[m] - rng_s[m] * RS).astype(np.int16)
        dv = (dst_s[m] - (c * RS + sc_s[m] * SW * P)).astype(np.float32)
        idx_flat = np.zeros(tot_slots, np.int16)
        dst_flat = np.full(tot_slots, -32000.0, np.float32)
        idx_flat[sl] = iv
        dst_flat[sl] = dv
        wrapped = np.zeros((16, tot_slots // 16), np.int16)
        for s in range(NSC):
            o = sc_slot_off[s]
            n = sc_slots[s]
            blk = idx_flat[o:o + n]
            i = np.arange(n)
            wrapped[i % 16, o // 16 + i // 16] = blk
        idxw.append(np.tile(wrapped, (8, 1)))
        dstw.append(np.ascontiguousarray(
            dst_flat.reshape(tot_chunks, P).T))

    # spans: per (sc, sc-local chunk) union over cores of touched windows
    wk = np.where(slot >= 0, slot // P, 0)
    w_in_sc = (dst_s - (core_s * RS + sc_s * SW * P)) // P
    spans = {}
    chunk_min = np.full(tot_chunks, 10 ** 9, np.int64)
    chunk_max = np.full(tot_chunks, -1, np.int64)
    np.minimum.at(chunk_min, slot // P, w_in_sc)
    np.maximum.at(chunk_max, slot // P, w_in_sc)

    sc_windows = [min(SW, WPC - s * SW) for s in range(NSC)]
    chunk_of = []  # per sc: list of (global chunk col, r, block)
    for s in range(NSC):
        lst = []
        for r in range(NCORES):
            base = (sc_slot_off[s] + run_off[s, r]) // P
            for b in range(L[s, r] // P):
                lst.append((int(base + b), r, b))
        chunk_of.append(lst)
        for cc, (gc, r, b) in enumerate(lst):
            lo, hi = chunk_min[gc], chunk_max[gc]
            if hi < 0:
                spans[(s, cc)] = []
            else:
                spans[(s, cc)] = list(range(int(lo), int(hi) + 1))

    return dict(L=L, sc_windows=sc_windows, idxw=idxw, dstw=dstw,
                spans=spans, chunk_of=chunk_of,
                sc_slot_off=sc_slot_off, sc_slots=sc_slots,
                tot_slots=tot_slots, tot_chunks=tot_chunks)


def _build_program(sched, wrap_niter=None):
    import concourse.bacc as bacc
    import concourse.mybir as mybir
    from concourse.tile import TileContext

    L = sched["L"]
    sc_windows = sched["sc_windows"]
    spans = sched["spans"]
    chunk_of = sched["chunk_of"]
    sc_slot_off = sched["sc_slot_off"]
    sc_slots = sched["sc_slots"]
    tot_slots = sched["tot_slots"]
    tot_chunks = sched["tot_chunks"]

    nc = bacc.Bacc("TRN2", target_bir_lowering=False, num_swdge_queues=4)
    z0 = nc.dram_tensor("z0", [NPAD, D], mybir.dt.bfloat16, kind="ExternalInput")
    idxw = nc.dram_tensor("idxw", [128, tot_slots // 16], mybir.dt.int16,
                          kind="ExternalInput")
    dstw = nc.dram_tensor("dstw", [P, tot_chunks], mybir.dt.float32,
                          kind="ExternalInput")
    disw = nc.dram_tensor("disw", [P, WPC], mybir.dt.float32, kind="ExternalInput")
    dis2w = nc.dram_tensor("dis2w", [P, WPC], mybir.dt.float32, kind="ExternalInput")
    outs = [nc.dram_tensor(f"out{l}", [RS, D], mybir.dt.float32,
                           kind="ExternalOutput") for l in range(3)]
    cc_in = [nc.dram_tensor(f"cc_in{l}", [RS, D], mybir.dt.bfloat16,
                            kind="Internal") for l in range(2)]
    cc_out = [nc.dram_tensor(f"cc_out{l}", [NPAD, D], mybir.dt.bfloat16,
                             kind="Internal", addr_space="Shared")
              for l in range(2)]

    with TileContext(nc) as tc:
        with tc.tile_pool(name="cpool", bufs=1) as cpool, \
             tc.tile_pool(name="ipool", bufs=2) as ipool, \
             tc.tile_pool(name="gpool", bufs=2) as gpool, \
             tc.tile_pool(name="mpool", bufs=4) as mpool, \
             tc.tile_pool(name="epool", bufs=3) as epool, \
             tc.tile_pool(name="pspool", bufs=2, space="PSUM") as pspool:
            iota_sb = cpool.tile([P, P], mybir.dt.bfloat16)
            nc.gpsimd.iota(iota_sb[:], pattern=[[1, P]], base=0,
                           channel_multiplier=0,
                           allow_small_or_imprecise_dtypes=True)
            dstw_sb = cpool.tile([P, tot_chunks], mybir.dt.float32)
            nc.sync.dma_start(out=dstw_sb[:], in_=dstw[:])
            disw_sb = cpool.tile([P, WPC], mybir.dt.float32)
            nc.sync.dma_start(out=disw_sb[:], in_=disw[:])
            dis2w_sb = cpool.tile([P, WPC], mybir.dt.float32)
            nc.sync.dma_start(out=dis2w_sb[:], in_=dis2w[:])

            def layer_body(l):
                    zsrc = z0 if l == 0 else cc_out[l - 1]
                    for s in range(NSC):
                        nsl = int(sc_slots[s])
                        o16 = int(sc_slot_off[s]) // 16
                        idx_sb = ipool.tile([128, nsl // 16], mybir.dt.int16,
                                            tag="idx")
                        nc.sync.dma_start(out=idx_sb[:],
                                          in_=idxw[:, o16:o16 + nsl // 16])
                        gs = []
                        for r in range(NCORES):
                            lr = int(L[s, r])
                            g_sb = gpool.tile([P, lr // P, D],
                                              mybir.dt.bfloat16, tag=f"g{r}")
                            gs.append(g_sb)
                            ro16 = int(np.concatenate([[0], np.cumsum(L[s])])[r]) // 16
                            nc.gpsimd.dma_gather(
                                g_sb[:],
                                zsrc[r * RS:(r + 1) * RS, :],
                                idx_sb[:, ro16:ro16 + lr // 16],
                                lr, lr, D, single_packet=False,
                                queue_num=r % 4,
                            )
                        scw = sc_windows[s]
                        ngrp = math.ceil(scw / WGRP)
                        for wg in range(ngrp):
                            glo = wg * WGRP
                            ghi = min(glo + WGRP, scw)
                            pst = {}
                            for wi in range(glo, ghi):
                                t = pspool.tile([P, D], mybir.dt.float32,
                                                tag=f"w{wi % WGRP}")
                                pst[wi] = t
                            # emission: (chunk, window) pairs; flags
                            pairs_by_w = {wi: [] for wi in range(glo, ghi)}
                            for cc, (gc, r, b) in enumerate(chunk_of[s]):
                                for wi in spans[(s, cc)]:
                                    if glo <= wi < ghi:
                                        pairs_by_w[wi].append(cc)
                            for wi in range(glo, ghi):
                                if not pairs_by_w[wi]:
                                    pairs_by_w[wi] = [0]  # dummy zero-M pair
                            emitted = {wi: 0 for wi in range(glo, ghi)}
                            for cc, (gc, r, b) in enumerate(chunk_of[s]):
                                for wi in range(glo, ghi):
                                    plist = pairs_by_w[wi]
                                    if cc not in plist:
                                        continue
                                    m_sb = mpool.tile([P, P], mybir.dt.bfloat16,
                                                      tag="m")
                                    nc.vector.tensor_scalar(
                                        out=m_sb[:], in0=iota_sb[:],
                                        scalar1=dstw_sb[:, gc:gc + 1],
                                        scalar2=float(-wi * P),
                                        op0=mybir.AluOpType.subtract,
                                        op1=mybir.AluOpType.is_equal,
                                    )
                                    k = emitted[wi]
                                    nc.tensor.matmul(
                                        out=pst[wi][:], lhsT=m_sb[:],
                                        rhs=gs[r][:, b, :],
                                        start=(k == 0),
                                        stop=(k == len(plist) - 1),
                                    )
                                    emitted[wi] += 1
                            for wi in range(glo, ghi):
                                wg_glob = s * SW + wi
                                row0 = wg_glob * P
                                emb_sb = epool.tile([P, D], mybir.dt.float32,
                                                    tag="emb")
                                nc.scalar.activation(
                                    out=emb_sb[:], in_=pst[wi][:],
                                    func=mybir.ActivationFunctionType.Copy,
                                    scale=disw_sb[:, wg_glob:wg_glob + 1],
                                )
                                nc.sync.dma_start(
                                    out=outs[l][row0:row0 + P, :], in_=emb_sb[:])
                                if l < 2:
                                    zn_sb = epool.tile([P, D],
                                                       mybir.dt.bfloat16,
                                                       tag="zn")
                                    nc.vector.tensor_scalar(
                                        out=zn_sb[:], in0=pst[wi][:],
                                        scalar1=dis2w_sb[:, wg_glob:wg_glob + 1],
                                        scalar2=None,
                                        op0=mybir.AluOpType.mult,
                                    )
                                    nc.sync.dma_start(
                                        out=cc_in[l][row0:row0 + P, :],
                                        in_=zn_sb[:])
            for l in range(3):
                if wrap_niter is None or wrap_niter == 1:
                    layer_body(l)
                else:
                    with tc.For_i(0, wrap_niter, 1):
                        layer_body(l)
                if l < 2:
                    nc.gpsimd.collective_compute(
                        kind="AllGather", op=mybir.AluOpType.bypass,
                        replica_groups=[list(range(NCORES))],
                        ins=[cc_in[l][:]], outs=[cc_out[l][:]],
                    )
    nc.compile()
    return nc


def _prepare(edge_index):
    src = np.asarray(edge_index[0], np.int64)
    dst = np.asarray(edge_index[1], np.int64)
    deg = np.bincount(dst, minlength=NPAD).astype(np.float32)
    dis = np.where(deg > 0, 1.0 / np.sqrt(np.maximum(deg, 1.0)), 0.0).astype(
        np.float32)
    sched = _build_schedule(src, dst)
    # per-core dis window tiles [128, 196]
    disw, dis2w = [], []
    for c in range(NCORES):
        dslice = dis[c * RS:(c + 1) * RS]
        disw.append(np.ascontiguousarray(dslice.reshape(WPC, P).T))
        dis2w.append(np.ascontiguousarray((dslice * dslice).reshape(WPC, P).T))
    return dis, sched, disw, dis2w


def kernel(user_int, item_int, user_geo, item_geo, edge_index):
    from concourse.bass_utils import run_bass_kernel_spmd

    user_int = np.asarray(user_int, np.float32)
    item_int = np.asarray(item_int, np.float32)
    user_geo = np.asarray(user_geo, np.float32)
    item_geo = np.asarray(item_geo, np.float32)
    edge_index = np.asarray(edge_index)

    key = ("prog", edge_index.tobytes()[:4096].__hash__(),
           int(edge_index.sum()))
    if key not in _cache:
        dis, sched, disw, dis2w = _prepare(edge_index)
        nc = _build_program(sched)
        _cache[key] = (dis, sched, disw, dis2w, nc)
    dis, sched, disw, dis2w, nc = _cache[key]

    X0 = np.concatenate(
        [np.concatenate([user_int, item_int], axis=0),
         np.concatenate([user_geo, item_geo], axis=0)], axis=1)  # [N, 128]
    z0 = np.zeros((NPAD, D), np.float32)
    z0[:N] = X0 * dis[:N, None]
    z0 = z0.astype(BF16)

    in_maps = []
    for c in range(NCORES):
        in_maps.append({
            "z0": z0,
            "idxw": sched["idxw"][c],
            "dstw": sched["dstw"][c],
            "disw": disw[c],
            "dis2w": dis2w[c],
        })
    res = run_bass_kernel_spmd(nc, in_maps, core_ids=list(range(NCORES)))

    emb_sum = np.concatenate(
        [res.results[c]["out0"] + res.results[c]["out1"] + res.results[c]["out2"]
         for c in range(NCORES)], axis=0)[:N]
    out = (X0 + emb_sum) / float((3 + 1) ** 2)
    return (np.ascontiguousarray(out[:N_USER, :64]),
            np.ascontiguousarray(out[N_USER:, :64]),
            np.ascontiguousarray(out[:N_USER, 64:]),
            np.ascontiguousarray(out[N_USER:, 64:]))

